# revision 32
# baseline (speedup 1.0000x reference)
"""GraphSAGE (2x SAGE-GCN conv + MLP head w/ BatchNorm) on 8 Trainium2 NeuronCores.

Sharding: nodes partitioned into 8 contiguous ranges (graph/data
parallel). h1 is exchanged via TWO bf16 AllGathers (rows [0,4096) and
[4096,6250) of every core's slice) so the second, smaller collective
and the first wave of layer-2 gathers overlap with compute; edges are
segregated per dst block by which half-table their src row lives in
(both tables < 32768 rows, so int16 SWDGE gather indices address them
directly). Layer-2 neighbor rows arrive via software-DGE dma_gather
(1024 descriptors/instruction, 4 queues, deep block prefetch).
Segment-sum is one-hot matmuls accumulating in fp32 PSUM; one-hots come
from DVE is_equal (odd blocks) or are host-streamed (even blocks, layer
1) to dodge the DVE broadcast 1x-mode wall. Self loops are never
materialized: the 2h term is one 2I @ h_own matmul into the same PSUM
accumulation. Batch stats accumulate per-block on the vector engine and
cross-core via a tiny AllReduce; BN folds into the final matvec, which
runs in transposed (zT) layout on the tensor engine.
"""
import sys

sys.path.insert(0, "/opt/trn_rl_repo")

import numpy as np
BF16 = np.float16

N = 50000
E = 800000
DIN, DH, MH = 64, 128, 200
EPS = 1e-5
NC = 8
NLOC = N // NC          # 6250
P = 128
NB = (NLOC + P - 1) // P  # 49 blocks (48 full + 1 of 106 rows)
LAST_ROWS = NLOC - (NB - 1) * P  # 106
NPAD = NB * P           # 6272
MH2 = MH - P            # 72
RSPLIT = 4096           # first-half rows of each core's slice: the int16
                        # max (8*4096 = 32768 rows) so AllGather #2 (the
                        # serial one) is as small as possible
BSZ = NLOC - RSPLIT     # second-half rows per core
NHALF_A = NC * RSPLIT   # rows in half-table A (< 32768: int16 ok)
NHALF_B = NC * BSZ      # rows in half-table B (< 32768: int16 ok)

import os as _os

GCHUNK = int(_os.environ.get("K_GCHUNK", "8"))
GSPBUFS = int(_os.environ.get("K_GSPBUFS", "11"))
FSPBUFS = int(_os.environ.get("K_FSPBUFS", "3"))
OHPBUFS = int(_os.environ.get("K_OHPBUFS", "3"))
SCRATCH = int(_os.environ.get("K_SCRATCH", "65536"))
PF_A = int(_os.environ.get("K_PF_A", "10"))
PF_B = int(_os.environ.get("K_PF_B", "5"))
if _os.environ.get("K_NOSTREAM"):
    STREAM1 = frozenset()
else:
    STREAM1 = frozenset(b for b in range(NB) if b % 2 == 0)


def _build_edge_layout(src, dst):
    """Per-core, per-dst-block edge tiling (real edges only). Within each
    block, edges are segregated by src half (owner-slice row < RSPLIT) so
    each half-table AllGather unlocks its segment's gathers. Tile counts
    are the max over cores so the SPMD program is identical on every
    core. Pad slots get gather index -1 (descriptor skipped)."""
    EA = src.size
    core = dst // NLOC
    rem = dst % NLOC
    dloc = rem % P
    blk = rem // P
    o = src // NLOC
    r = src % NLOC
    isB = (r >= RSPLIT).astype(np.int64)
    grow = np.where(isB == 1, o * BSZ + (r - RSPLIT), o * RSPLIT + r)

    cnt = np.zeros((2, NC, NB), np.int64)
    np.add.at(cnt, (isB, core, blk), 1)
    ntA = np.maximum(1, (cnt[0].max(axis=0) + P - 1) // P)  # [NB]
    ntB = np.maximum(1, (cnt[1].max(axis=0) + P - 1) // P)  # [NB]
    n_tiles = ntA + ntB
    tile_of_block = np.zeros(NB + 1, np.int64)
    tile_of_block[1:] = np.cumsum(n_tiles)
    T = int(tile_of_block[-1])

    gidx = np.full((NC, P, T), 0, np.int32)    # pads gather row 0
    gnode = np.zeros((NC, P, T), np.int32)     # original src id (fexp)
    dlocT = np.full((NC, P, T), -1.0, np.float32)
    order = np.lexsort((dloc, isB, blk, core))
    s_row = grow[order].astype(np.int32)
    s_node = src[order].astype(np.int32)
    s_core = core[order]
    s_blk = blk[order]
    s_dloc = dloc[order]
    s_half = isB[order]
    flat_cnt = cnt.transpose(1, 2, 0).ravel()  # (core, blk, half) order
    starts = np.zeros(NC * NB * 2, np.int64)
    starts[1:] = np.cumsum(flat_cnt)[:-1]
    grp_start = starts.reshape(NC, NB, 2)
    pos_in_grp = np.arange(EA) - grp_start[s_core, s_blk, s_half]
    seg_base = np.where(s_half == 1, ntA[s_blk] * P, 0)
    pos = pos_in_grp + seg_base
    t_glob = tile_of_block[s_blk] + pos // P
    p_idx = pos % P
    gidx[s_core, p_idx, t_glob] = s_row
    gnode[s_core, p_idx, t_glob] = s_node
    dlocT[s_core, p_idx, t_glob] = s_dloc.astype(np.float32)
    return n_tiles, ntA, tile_of_block, T, gidx, gnode, dlocT


def build_program(n_tiles, ntA, tob, T):
    import concourse.bacc as bacc
    import concourse.bass as bass
    import concourse.tile as tile
    import concourse.mybir as mybir

    f32 = mybir.dt.float32
    bf16 = mybir.dt.float16
    i16 = mybir.dt.int16
    AF = mybir.ActivationFunctionType
    OP = mybir.AluOpType
    core_ids = list(range(NC))
    NTBMAX = int(max(n_tiles))
    # L1 blocks whose one-hot tiles stream from DRAM (rest: DVE is_equal)
    soff = {}
    ts = 0
    for b in range(NB):
        if b in STREAM1:
            soff[b] = ts
            ts += int(n_tiles[b])
    TS = max(ts, 1)

    nc = bacc.Bacc(None, target_bir_lowering=False, debug=False,
                   dynamic_dma_scratch_size=SCRATCH, num_swdge_queues=4)

    # ---- I/O ----
    fexp_d = nc.dram_tensor("fexp", [P, T * DIN], bf16, kind="ExternalInput")
    gidx_d = nc.dram_tensor("gidx", [P, T * 8], i16, kind="ExternalInput")
    dloc_d = nc.dram_tensor("dloc", [P, T], bf16, kind="ExternalInput")
    inv2_d = nc.dram_tensor("inv2", [P, NB], f32, kind="ExternalInput")
    fown_d = nc.dram_tensor("fown", [P, NB * DIN], bf16, kind="ExternalInput")
    w1_d = nc.dram_tensor("w1", [DIN, DH], bf16, kind="ExternalInput")
    w2_d = nc.dram_tensor("w2", [DH, DH], bf16, kind="ExternalInput")
    wm1_d = nc.dram_tensor("wm1", [DH, MH], bf16, kind="ExternalInput")
    b1_d = nc.dram_tensor("b1c", [DH, 1], f32, kind="ExternalInput")
    b2_d = nc.dram_tensor("b2c", [DH, 1], f32, kind="ExternalInput")
    # packed per-partition column params:
    # 0=bm1[:128] 1=bm1[128:] 2=gamma[:128] 3=gamma[128:]
    # 4=beta[:128] 5=beta[128:] 6=wm2[:128] 7=wm2[128:]
    bnpk_d = nc.dram_tensor("bnpk", [P, 8], f32, kind="ExternalInput")
    bm2_d = nc.dram_tensor("bm2s", [1, 1], f32, kind="ExternalInput")
    iota_d = nc.dram_tensor("iota", [P, P], bf16, kind="ExternalInput")
    identb_d = nc.dram_tensor("identb", [P, P], bf16, kind="ExternalInput")
    ident2_d = nc.dram_tensor("ident2", [P, P], bf16, kind="ExternalInput")
    ohs_d = nc.dram_tensor("ohs", [P, TS * P], bf16, kind="ExternalInput")
    identf_d = nc.dram_tensor("identf", [P, P], f32, kind="ExternalInput")
    out_d = nc.dram_tensor("out", [1, NPAD], f32, kind="ExternalOutput")

    # internal DRAM
    sliceA = nc.dram_tensor("sliceA", [RSPLIT, DH], bf16)
    sliceB = nc.dram_tensor("sliceB", [BSZ, DH], bf16)
    h1A = nc.dram_tensor("h1A", [NHALF_A, DH], bf16, addr_space="Shared")
    h1B = nc.dram_tensor("h1B", [NHALF_B, DH], bf16, addr_space="Shared")
    stats_in = nc.dram_tensor("stats_in", [P, 4], f32)
    stats_out = nc.dram_tensor("stats_out", [P, 4], f32, addr_space="Shared")

    with tile.TileContext(nc) as tc:
        with tc.tile_pool(name="persist", bufs=1) as pp, \
             tc.tile_pool(name="fstream", bufs=FSPBUFS) as fsp, \
             tc.tile_pool(name="ohpool", bufs=OHPBUFS) as ohp, \
             tc.tile_pool(name="gpool", bufs=GSPBUFS) as gsp, \
             tc.tile_pool(name="tmp", bufs=3) as tp, \
             tc.tile_pool(name="pagg", bufs=2, space="PSUM") as pagg, \
             tc.tile_pool(name="ptrp", bufs=2, space="PSUM") as ptrp, \
             tc.tile_pool(name="pwz", bufs=2, space="PSUM") as pwz, \
             tc.tile_pool(name="pfin", bufs=2, space="PSUM") as pfin:

            # ---- persistent tiles ----
            gidx_t = pp.tile([P, T * 8], i16)
            dloc_t = pp.tile([P, T], bf16)
            inv2_t = pp.tile([P, NB], f32)
            fown_t = pp.tile([P, NB, DIN], bf16)
            w1_t = pp.tile([DIN, DH], bf16)
            w2_t = pp.tile([DH, DH], bf16)
            wm1_t = pp.tile([DH, MH], bf16)
            b1_t = pp.tile([DH, 1], f32)
            b2_t = pp.tile([DH, 1], f32)
            bnpk_t = pp.tile([P, 8], f32)
            iota_t = pp.tile([P, P], bf16)
            identb_t = pp.tile([P, P], bf16)
            ident2_t = pp.tile([P, P], bf16)
            identf_t = pp.tile([P, P], f32)
            h1own_t = pp.tile([P, NB, DH], bf16)
            zT1_t = pp.tile([P, NB, P], bf16)
            zT2_t = pp.tile([MH2, NB, P], bf16)
            stacc_t = pp.tile([P, 4], f32)
            eps_t = pp.tile([P, 1], f32)
            invN_t = pp.tile([P, 1], f32)
            nc.vector.memset(eps_t[:], EPS)
            nc.vector.memset(invN_t[:], 1.0 / N)
            nc.vector.memset(stacc_t[:], 0.0)

            nc.sync.dma_start(gidx_t[:], gidx_d[:])
            nc.sync.dma_start(dloc_t[:], dloc_d[:])
            nc.sync.dma_start(inv2_t[:], inv2_d[:])
            nc.scalar.dma_start(fown_t[:], fown_d.rearrange(
                "p (b d) -> p b d", d=DIN))
            nc.sync.dma_start(w1_t[:], w1_d[:])
            nc.sync.dma_start(w2_t[:], w2_d[:])
            nc.sync.dma_start(wm1_t[:], wm1_d[:])
            nc.sync.dma_start(b1_t[:], b1_d[:])
            nc.sync.dma_start(b2_t[:], b2_d[:])
            nc.sync.dma_start(bnpk_t[:], bnpk_d[:])
            nc.sync.dma_start(iota_t[:], iota_d[:])
            nc.sync.dma_start(identb_t[:], identb_d[:])
            nc.sync.dma_start(ident2_t[:], ident2_d[:])
            nc.sync.dma_start(identf_t[:], identf_d[:])

            fexp_r = fexp_d.rearrange("p (t d) -> p t d", d=DIN)
            ohs_r = ohs_d.rearrange("p (t j) -> p t j", j=P)

            def onehot(b, eng):
                t0, t1 = int(tob[b]), int(tob[b + 1])
                ntb = t1 - t0
                oh = ohp.tile([P, NTBMAX, P], bf16, tag="oh")
                eng.tensor_tensor(
                    out=oh[:, :ntb, :],
                    in0=dloc_t[:, t0:t1].unsqueeze(2).to_broadcast(
                        [P, ntb, P]),
                    in1=iota_t[:].unsqueeze(1).to_broadcast([P, ntb, P]),
                    op=OP.is_equal)
                return oh

            # ---- layer 1: stream pre-gathered features ----
            for b in range(NB):
                rows_b = P if b < NB - 1 else LAST_ROWS
                t0, t1 = int(tob[b]), int(tob[b + 1])
                ntb = t1 - t0
                rt = fsp.tile([P, NTBMAX, DIN], bf16, tag="ft")
                deng = nc.sync if b % 2 == 0 else nc.scalar
                deng.dma_start(rt[:, :ntb, :], fexp_r[:, t0:t1, :])
                if b in STREAM1:
                    oh = ohp.tile([P, NTBMAX, P], bf16, tag="oh")
                    oeng = nc.scalar if b % 2 == 0 else nc.sync
                    oeng.dma_start(oh[:, :ntb, :],
                                   ohs_r[:, soff[b]:soff[b] + ntb, :])
                else:
                    oh = onehot(b, nc.vector)
                pm = pagg.tile([P, DH], f32, tag="pm")
                for ti in range(ntb):
                    nc.tensor.matmul(out=pm[:, :DIN], lhsT=oh[:, ti, :],
                                     rhs=rt[:, ti, :],
                                     start=(ti == 0), stop=False)
                nc.tensor.matmul(out=pm[:, :DIN], lhsT=ident2_t[:],
                                 rhs=fown_t[:, b, :], start=False, stop=True)
                hn = tp.tile([P, DIN], bf16, tag="hn")
                nc.scalar.activation(hn[:], pm[:, :DIN], AF.Copy,
                                     scale=inv2_t[:, b:b + 1])
                ptt = ptrp.tile([P, P], bf16, tag="ptt")
                nc.tensor.transpose(out=ptt[:DIN, :], in_=hn[:],
                                    identity=identb_t[:])
                hnT = tp.tile([DIN, P], bf16, tag="hnT")
                nc.scalar.activation(hnT[:], ptt[:DIN, :], AF.Copy)
                pww = pwz.tile([P, MH + P], f32, tag="pwz")
                nc.tensor.matmul(out=pww[:, MH:], lhsT=w1_t[:], rhs=hnT[:],
                                 start=True, stop=True)
                hT = tp.tile([DH, P], bf16, tag="hT")
                nc.scalar.activation(hT[:], pww[:, MH:], AF.Relu,
                                     bias=b1_t[:])
                pt2 = ptrp.tile([P, P], bf16, tag="ptt")
                nc.tensor.transpose(out=pt2[:], in_=hT[:],
                                    identity=identb_t[:])
                nc.scalar.activation(h1own_t[:, b, :], pt2[:], AF.Copy)
                # rows [b*128, b*128+rows_b) of the slice, split at RSPLIT
                lo = b * P
                hi = lo + rows_b
                if hi <= RSPLIT:
                    nc.sync.dma_start(sliceA[lo:hi, :],
                                      h1own_t[:rows_b, b, :])
                elif lo >= RSPLIT:
                    nc.sync.dma_start(sliceB[lo - RSPLIT:hi - RSPLIT, :],
                                      h1own_t[:rows_b, b, :])
                else:
                    cut = RSPLIT - lo
                    nc.sync.dma_start(sliceA[lo:RSPLIT, :],
                                      h1own_t[:cut, b, :])
                    nc.sync.dma_start(sliceB[0:hi - RSPLIT, :],
                                      h1own_t[cut:rows_b, b, :])
                if hi >= RSPLIT and lo < RSPLIT:
                    # first half of every core's slice is complete
                    nc.gpsimd.collective_compute(
                        "AllGather", mybir.AluOpType.bypass,
                        replica_groups=[core_ids],
                        ins=[sliceA[:]], outs=[h1A[:]],
                    )

            # ---- layer 2: gather h1 rows, aggregate, fused MLP hidden ----
            rts = {}
            gq = [0]

            def gather_seg(b, s0, s1, tab):
                rt = rts[b]
                t0 = int(tob[b])
                nt = s1 - s0
                k = (nt + GCHUNK - 1) // GCHUNK
                base, rem = divmod(nt, k)
                c0 = s0
                for j in range(k):  # balanced chunk sizes
                    c1 = c0 + base + (1 if j < rem else 0)
                    ni = (c1 - c0) * P
                    nc.gpsimd.dma_gather(
                        out_ap=rt[:, c0:c1, :], in_ap=tab,
                        idxs_ap=gidx_t[:, 8 * (t0 + c0):8 * (t0 + c1)],
                        num_idxs=ni, num_idxs_reg=ni,
                        elem_size=DH,
                        queue_num=gq[0] % 4,
                    )
                    gq[0] += 1
                    c0 = c1

            def gather_a(b):
                rts[b] = gsp.tile([P, NTBMAX, DH], bf16, tag="gt",
                                  name=f"rt{b}")
                gather_seg(b, 0, int(ntA[b]), h1A[:])

            def gather_b(b):
                t0, t1 = int(tob[b]), int(tob[b + 1])
                gather_seg(b, int(ntA[b]), t1 - t0, h1B[:])

            # A-prefetch issues as soon as AG1 lands (gpsimd idle in L1);
            # the AG2 trigger goes after a few A-gathers: late enough not
            # to stall them, early enough to fire soon after sliceB lands
            for b in range(min(PF_A, NB)):
                if b == 4:
                    nc.gpsimd.collective_compute(
                        "AllGather", mybir.AluOpType.bypass,
                        replica_groups=[core_ids],
                        ins=[sliceB[:]], outs=[h1B[:]],
                    )
                gather_a(b)
            for b in range(min(PF_B, NB)):
                gather_b(b)

            for b in range(NB):
                if b + PF_A < NB:
                    gather_a(b + PF_A)
                if b + PF_B < NB:
                    gather_b(b + PF_B)
                t0, t1 = int(tob[b]), int(tob[b + 1])
                ntb = t1 - t0
                rt = rts.pop(b)
                oh = onehot(b, nc.vector)
                pm = pagg.tile([P, DH], f32, tag="pm")
                for ti in range(ntb):
                    nc.tensor.matmul(out=pm[:], lhsT=oh[:, ti, :],
                                     rhs=rt[:, ti, :],
                                     start=(ti == 0), stop=False)
                nc.tensor.matmul(out=pm[:], lhsT=ident2_t[:],
                                 rhs=h1own_t[:, b, :], start=False, stop=True)
                hn = tp.tile([P, DH], bf16, tag="hn2")
                nc.scalar.activation(hn[:], pm[:], AF.Copy,
                                     scale=inv2_t[:, b:b + 1])
                ptt = ptrp.tile([P, P], bf16, tag="ptt")
                nc.tensor.transpose(out=ptt[:], in_=hn[:],
                                    identity=identb_t[:])
                hnT = tp.tile([DH, P], bf16, tag="hnT2")
                nc.scalar.activation(hnT[:], ptt[:], AF.Copy)
                pww = pwz.tile([P, MH + P], f32, tag="pwz")
                nc.tensor.matmul(out=pww[:, MH:], lhsT=w2_t[:], rhs=hnT[:],
                                 start=True, stop=True)
                h2T = tp.tile([DH, P], bf16, tag="h2T")
                nc.scalar.activation(h2T[:], pww[:, MH:], AF.Relu,
                                     bias=b2_t[:])
                # fused MLP hidden in transposed layout:
                # zT[m, p] = relu(sum_d wm1[d, m] h2T[d, p] + bm1[m])
                pz = pwz.tile([P, MH + P], f32, tag="pwz")
                nc.tensor.matmul(out=pz[:, :P], lhsT=wm1_t[:, :P],
                                 rhs=h2T[:], start=True, stop=True)
                nc.tensor.matmul(out=pz[:MH2, P:2 * P], lhsT=wm1_t[:, P:],
                                 rhs=h2T[:], start=True, stop=True)
                nc.scalar.activation(zT1_t[:, b, :], pz[:, :P], AF.Relu,
                                     bias=bnpk_t[:, 0:1])
                nc.scalar.activation(zT2_t[:, b, :], pz[:MH2, P:2 * P],
                                     AF.Relu, bias=bnpk_t[:MH2, 1:2])
                if b == NB - 1:
                    # zero pad columns so batch stats stay clean
                    nc.vector.memset(zT1_t[:, b, LAST_ROWS:], 0.0)
                    nc.vector.memset(zT2_t[:, b, LAST_ROWS:], 0.0)
                # incremental batch stats: sum(z), sum(z^2) over this block
                sqb = tp.tile([P, 2 * P], bf16, tag="sqb")
                nc.scalar.activation(sqb[:, :P], zT1_t[:, b, :], AF.Square)
                nc.scalar.activation(sqb[:MH2, P:], zT2_t[:, b, :],
                                     AF.Square)
                rb = tp.tile([P, 4], f32, tag="rb")
                nc.vector.tensor_reduce(out=rb[:, 0:1], in_=zT1_t[:, b, :],
                                        axis=mybir.AxisListType.X, op=OP.add)
                nc.vector.tensor_reduce(out=rb[:, 1:2], in_=sqb[:, :P],
                                        axis=mybir.AxisListType.X, op=OP.add)
                nc.vector.tensor_reduce(out=rb[:MH2, 2:3],
                                        in_=zT2_t[:, b, :],
                                        axis=mybir.AxisListType.X, op=OP.add)
                nc.vector.tensor_reduce(out=rb[:MH2, 3:4],
                                        in_=sqb[:MH2, P:],
                                        axis=mybir.AxisListType.X, op=OP.add)
                nc.vector.tensor_tensor(out=stacc_t[:, 0:2],
                                        in0=stacc_t[:, 0:2],
                                        in1=rb[:, 0:2], op=OP.add)
                nc.vector.tensor_tensor(out=stacc_t[:MH2, 2:4],
                                        in0=stacc_t[:MH2, 2:4],
                                        in1=rb[:MH2, 2:4], op=OP.add)

            nc.sync.dma_start(stats_in[:], stacc_t[:])
            nc.gpsimd.collective_compute(
                "AllReduce", mybir.AluOpType.add,
                replica_groups=[core_ids],
                ins=[stats_in[:]], outs=[stats_out[:]],
            )
            gst = tp.tile([P, 4], f32, tag="gst")
            nc.sync.dma_start(gst[:], stats_out[:])

            # ---- fold BN into the final matvec (per-partition columns) ----
            mu = tp.tile([P, 2], f32, tag="mu")
            var = tp.tile([P, 2], f32, tag="var")
            scl = tp.tile([P, 2], f32, tag="scl")
            wp = tp.tile([P, 2], bf16, tag="wp")
            ws = tp.tile([P, 2], f32, tag="ws")
            nc.vector.tensor_tensor(out=mu[:, 0:1], in0=gst[:, 0:1],
                                    in1=invN_t[:], op=OP.mult)
            nc.vector.tensor_tensor(out=mu[:, 1:2], in0=gst[:, 2:3],
                                    in1=invN_t[:], op=OP.mult)
            nc.vector.tensor_tensor(out=var[:, 0:1], in0=gst[:, 1:2],
                                    in1=invN_t[:], op=OP.mult)
            nc.vector.tensor_tensor(out=var[:, 1:2], in0=gst[:, 3:4],
                                    in1=invN_t[:], op=OP.mult)
            mu2 = tp.tile([P, 2], f32, tag="mu2")
            nc.vector.tensor_tensor(out=mu2[:], in0=mu[:], in1=mu[:],
                                    op=OP.mult)
            nc.vector.tensor_tensor(out=var[:], in0=var[:], in1=mu2[:],
                                    op=OP.subtract)
            nc.scalar.activation(var[:], var[:], AF.Sqrt, bias=eps_t[:])
            rstd = tp.tile([P, 2], f32, tag="rstd")
            nc.vector.reciprocal(rstd[:], var[:])
            # scale = gamma * rstd ; shift = beta - mu * scale
            nc.vector.tensor_tensor(out=scl[:, 0:1], in0=bnpk_t[:, 2:3],
                                    in1=rstd[:, 0:1], op=OP.mult)
            nc.vector.tensor_tensor(out=scl[:, 1:2], in0=bnpk_t[:, 3:4],
                                    in1=rstd[:, 1:2], op=OP.mult)
            msc = tp.tile([P, 2], f32, tag="msc")
            nc.vector.tensor_tensor(out=msc[:], in0=mu[:], in1=scl[:],
                                    op=OP.mult)
            shf = tp.tile([P, 2], f32, tag="shf")
            nc.vector.tensor_tensor(out=shf[:, 0:1], in0=bnpk_t[:, 4:5],
                                    in1=msc[:, 0:1], op=OP.subtract)
            nc.vector.tensor_tensor(out=shf[:, 1:2], in0=bnpk_t[:, 5:6],
                                    in1=msc[:, 1:2], op=OP.subtract)
            # w' = wm2 * scale (bf16 for matmul); ws = wm2 * shift (f32)
            wpf = tp.tile([P, 2], f32, tag="wpf")
            nc.vector.tensor_tensor(out=wpf[:, 0:1], in0=bnpk_t[:, 6:7],
                                    in1=scl[:, 0:1], op=OP.mult)
            nc.vector.tensor_tensor(out=wpf[:, 1:2], in0=bnpk_t[:, 7:8],
                                    in1=scl[:, 1:2], op=OP.mult)
            nc.scalar.activation(wp[:], wpf[:], AF.Copy)
            nc.vector.tensor_tensor(out=ws[:, 0:1], in0=bnpk_t[:, 6:7],
                                    in1=shf[:, 0:1], op=OP.mult)
            nc.vector.tensor_tensor(out=ws[:, 1:2], in0=bnpk_t[:, 7:8],
                                    in1=shf[:, 1:2], op=OP.mult)
            # (rows MH2: of ws lane 2 are zero by construction: bnpk pads)
            # b' = sum_m ws + bm2 : transpose columns to rows, reduce twice
            pts = pfin.tile([P, 4 * P], f32, tag="pfin")
            nc.tensor.transpose(out=pts[:2, :P], in_=ws[:],
                                identity=identf_t[:])
            wsrow = tp.tile([2, P], f32, tag="wsrow")
            nc.scalar.activation(wsrow[:], pts[:2, :P], AF.Copy)
            ssum = tp.tile([2, 1], f32, tag="ssum")
            nc.vector.tensor_reduce(out=ssum[:], in_=wsrow[:],
                                    axis=mybir.AxisListType.X, op=OP.add)
            pts2 = pfin.tile([P, 4 * P], f32, tag="pfin")
            nc.tensor.transpose(out=pts2[:1, :2], in_=ssum[:],
                                identity=identf_t[:2, :2])
            ssrow = tp.tile([1, 2], f32, tag="ssrow")
            nc.scalar.activation(ssrow[:], pts2[:1, :2], AF.Copy)
            tot = tp.tile([1, 1], f32, tag="tot")
            nc.vector.tensor_reduce(out=tot[:], in_=ssrow[:],
                                    axis=mybir.AxisListType.X, op=OP.add)
            bm2_t = tp.tile([1, 1], f32, tag="bm2t")
            nc.sync.dma_start(bm2_t[:], bm2_d[:])
            bpr = tp.tile([1, 1], f32, tag="bpr")
            nc.vector.tensor_tensor(out=bpr[:], in0=tot[:], in1=bm2_t[:],
                                    op=OP.add)

            # ---- final: sigmoid(w1'.zT1 + w2'.zT2 + b') on tensor eng ----
            zT1f = zT1_t[:].rearrange("m b p -> m (b p)")
            zT2f = zT2_t[:].rearrange("m b p -> m (b p)")
            CG = 4 * P  # 512 output columns per group
            ngrp = (NPAD + CG - 1) // CG
            for g in range(ngrp):
                c0 = g * CG
                c1 = min(c0 + CG, NPAD)
                cw = c1 - c0
                po = pfin.tile([P, 4 * P], f32, tag="pfin")
                nc.tensor.matmul(out=po[0:1, :cw], lhsT=wp[:, 0:1],
                                 rhs=zT1f[:, c0:c1], start=True, stop=False)
                nc.tensor.matmul(out=po[0:1, :cw], lhsT=wp[:MH2, 1:2],
                                 rhs=zT2f[:, c0:c1], start=False, stop=True)
                orow = tp.tile([1, CG], f32, tag="orow")
                nc.scalar.activation(orow[0:1, :cw], po[0:1, :cw],
                                     AF.Sigmoid, bias=bpr[:])
                nc.sync.dma_start(out_d[0:1, c0:c1], orow[0:1, :cw])

    nc.compile()
    return nc


# module-level cache of (program, layout) keyed by edge-structure hash
_CACHE = {}


def kernel(features, W1, b1, W2, b2, Wm1, bm1, gamma, beta, Wm2, bm2, src, dst):
    from concourse.bass_utils import run_bass_kernel_spmd

    features = np.asarray(features, np.float32)
    src = np.asarray(src, np.int64)
    dst = np.asarray(dst, np.int64)

    key = (int(src[:1000].sum()), int(dst[:1000].sum()), E)
    if key not in _CACHE:
        n_tiles, ntA, tob, T, gidx, gnode, dlocT = _build_edge_layout(src, dst)
        nc = build_program(n_tiles, ntA, tob, T)
        _CACHE[key] = (nc, n_tiles, ntA, tob, T, gidx, gnode, dlocT)
    nc, n_tiles, ntA, tob, T, gidx, gnode, dlocT = _CACHE[key]

    deg = np.bincount(dst, minlength=N).astype(np.float32)
    inv2 = (1.0 / (deg + 2.0)).astype(np.float32)
    features_bf = features.astype(BF16)

    iota = np.tile(np.arange(P, dtype=np.float32), (P, 1)).astype(BF16)
    identb = np.eye(P, dtype=np.float32).astype(BF16)
    ident2 = (2.0 * np.eye(P, dtype=np.float32)).astype(BF16)
    identf = np.eye(P, dtype=np.float32)
    jcols = np.arange(P, dtype=np.float32)

    W1b = np.asarray(W1, np.float32).astype(BF16)
    W2b = np.asarray(W2, np.float32).astype(BF16)
    Wm1b = np.asarray(Wm1, np.float32).astype(BF16)
    bm1f = np.asarray(bm1, np.float32).reshape(MH)
    gamf = np.asarray(gamma, np.float32).reshape(MH)
    betf = np.asarray(beta, np.float32).reshape(MH)
    wm2f = np.asarray(Wm2, np.float32).reshape(MH)
    bnpk = np.zeros((P, 8), np.float32)
    for i, v in enumerate((bm1f, gamf, betf, wm2f)):
        bnpk[:, 2 * i] = v[:P]
        bnpk[:MH2, 2 * i + 1] = v[P:]

    in_maps = []
    for c in range(NC):
        lo = c * NLOC
        fexp = features_bf[gnode[c]].reshape(P, T * DIN)
        # dma_gather int16 indices: position i=t*128+p at [16-wrap], x8 replicas
        flat = gidx[c].astype(np.int64).T.reshape(-1)  # i = t*128+p
        wrapped = flat.reshape(T * 8, 16).T.astype(np.int16)  # [16, T*8]
        gidx16 = np.ascontiguousarray(np.tile(wrapped, (8, 1)))  # [128, T*8]
        inv2p = np.zeros(NPAD, np.float32)
        inv2p[:NLOC] = inv2[lo:lo + NLOC]
        inv2T = np.ascontiguousarray(inv2p.reshape(NB, P).T)
        fownp = np.zeros((NPAD, DIN), BF16)
        fownp[:NLOC] = features_bf[lo:lo + NLOC]
        fown = np.ascontiguousarray(
            fownp.reshape(NB, P, DIN).transpose(1, 0, 2).reshape(P, NB * DIN))
        oh_parts = []
        for b in range(NB):
            if b in STREAM1:
                t0, t1 = int(tob[b]), int(tob[b + 1])
                ohb = (dlocT[c][:, t0:t1, None] == jcols[None, None, :])
                oh_parts.append(ohb.astype(BF16).reshape(P, -1))
        if oh_parts:
            ohs = np.ascontiguousarray(np.concatenate(oh_parts, axis=1))
        else:
            ohs = np.zeros((P, P), BF16)

        in_maps.append({
            "fexp": np.ascontiguousarray(fexp),
            "gidx": gidx16,
            "dloc": np.ascontiguousarray(dlocT[c].astype(BF16)),
            "inv2": inv2T,
            "fown": fown,
            "ohs": ohs,
            "w1": W1b,
            "w2": W2b,
            "wm1": Wm1b,
            "b1c": np.asarray(b1, np.float32).reshape(DH, 1),
            "b2c": np.asarray(b2, np.float32).reshape(DH, 1),
            "bnpk": bnpk,
            "bm2s": np.asarray(bm2, np.float32).reshape(1, 1),
            "iota": iota,
            "identb": identb,
            "ident2": ident2,
            "identf": identf,
        })

    res = run_bass_kernel_spmd(nc, in_maps, list(range(NC)))
    global _LAST
    _LAST = res
    out = np.concatenate(
        [res.results[c]["out"].reshape(-1)[:NLOC] for c in range(NC)], axis=0)
    return out.reshape(N, 1).astype(np.float32)


_LAST = None


# revision 33
# speedup vs baseline: 1.0976x; 1.0976x over previous
"""GraphSAGE (2x SAGE-GCN conv + MLP head w/ BatchNorm) on 8 Trainium2 NeuronCores.

Sharding: nodes partitioned into 8 contiguous ranges (graph/data
parallel). h1 is exchanged via TWO bf16 AllGathers (rows [0,4096) and
[4096,6250) of every core's slice) so the second, smaller collective
and the first wave of layer-2 gathers overlap with compute; edges are
segregated per dst block by which half-table their src row lives in
(both tables < 32768 rows, so int16 SWDGE gather indices address them
directly). Layer-2 neighbor rows arrive via software-DGE dma_gather
(1024 descriptors/instruction, 4 queues, deep block prefetch).
Segment-sum is one-hot matmuls accumulating in fp32 PSUM; one-hots come
from DVE is_equal (odd blocks) or are host-streamed (even blocks, layer
1) to dodge the DVE broadcast 1x-mode wall. Self loops are never
materialized: the 2h term is one 2I @ h_own matmul into the same PSUM
accumulation. Batch stats accumulate per-block on the vector engine and
cross-core via a tiny AllReduce; BN folds into the final matvec, which
runs in transposed (zT) layout on the tensor engine.
"""
import sys

sys.path.insert(0, "/opt/trn_rl_repo")

import numpy as np
BF16 = np.float16

N = 50000
E = 800000
DIN, DH, MH = 64, 128, 200
EPS = 1e-5
NC = 8
NLOC = N // NC          # 6250
P = 128
NB = (NLOC + P - 1) // P  # 49 blocks (48 full + 1 of 106 rows)
LAST_ROWS = NLOC - (NB - 1) * P  # 106
NPAD = NB * P           # 6272
MH2 = MH - P            # 72
RSPLIT = 4096           # first-half rows of each core's slice: the int16
                        # max (8*4096 = 32768 rows) so AllGather #2 (the
                        # serial one) is as small as possible
BSZ = NLOC - RSPLIT     # second-half rows per core
NHALF_A = NC * RSPLIT   # rows in half-table A (< 32768: int16 ok)
NHALF_B = NC * BSZ      # rows in half-table B (< 32768: int16 ok)

import os as _os

GCHUNK = int(_os.environ.get("K_GCHUNK", "8"))
GSPBUFS = int(_os.environ.get("K_GSPBUFS", "11"))
FSPBUFS = int(_os.environ.get("K_FSPBUFS", "3"))
OHPBUFS = int(_os.environ.get("K_OHPBUFS", "3"))
SCRATCH = int(_os.environ.get("K_SCRATCH", "65536"))
PF_A = int(_os.environ.get("K_PF_A", "10"))
PF_B = int(_os.environ.get("K_PF_B", "5"))
if _os.environ.get("K_NOSTREAM"):
    STREAM1 = frozenset()
else:
    STREAM1 = frozenset(b for b in range(NB) if b % 2 == 0)


def _build_edge_layout(src, dst):
    """Per-core, per-dst-block edge tiling (real edges only). Within each
    block, edges are segregated by src half (owner-slice row < RSPLIT) so
    each half-table AllGather unlocks its segment's gathers. Tile counts
    are the max over cores so the SPMD program is identical on every
    core. Pad slots get gather index -1 (descriptor skipped)."""
    EA = src.size
    core = dst // NLOC
    rem = dst % NLOC
    dloc = rem % P
    blk = rem // P
    o = src // NLOC
    r = src % NLOC
    isB = (r >= RSPLIT).astype(np.int64)
    grow = np.where(isB == 1, o * BSZ + (r - RSPLIT), o * RSPLIT + r)

    cnt = np.zeros((2, NC, NB), np.int64)
    np.add.at(cnt, (isB, core, blk), 1)
    ntA = np.maximum(1, (cnt[0].max(axis=0) + P - 1) // P)  # [NB]
    ntB = np.maximum(1, (cnt[1].max(axis=0) + P - 1) // P)  # [NB]
    n_tiles = ntA + ntB
    tile_of_block = np.zeros(NB + 1, np.int64)
    tile_of_block[1:] = np.cumsum(n_tiles)
    T = int(tile_of_block[-1])

    gidx = np.full((NC, P, T), 0, np.int32)    # pads gather row 0
    gnode = np.zeros((NC, P, T), np.int32)     # original src id (fexp)
    dlocT = np.full((NC, P, T), -1.0, np.float32)
    order = np.lexsort((dloc, isB, blk, core))
    s_row = grow[order].astype(np.int32)
    s_node = src[order].astype(np.int32)
    s_core = core[order]
    s_blk = blk[order]
    s_dloc = dloc[order]
    s_half = isB[order]
    flat_cnt = cnt.transpose(1, 2, 0).ravel()  # (core, blk, half) order
    starts = np.zeros(NC * NB * 2, np.int64)
    starts[1:] = np.cumsum(flat_cnt)[:-1]
    grp_start = starts.reshape(NC, NB, 2)
    pos_in_grp = np.arange(EA) - grp_start[s_core, s_blk, s_half]
    seg_base = np.where(s_half == 1, ntA[s_blk] * P, 0)
    pos = pos_in_grp + seg_base
    t_glob = tile_of_block[s_blk] + pos // P
    p_idx = pos % P
    gidx[s_core, p_idx, t_glob] = s_row
    gnode[s_core, p_idx, t_glob] = s_node
    dlocT[s_core, p_idx, t_glob] = s_dloc.astype(np.float32)
    return n_tiles, ntA, tile_of_block, T, gidx, gnode, dlocT


def build_program(n_tiles, ntA, tob, T):
    import concourse.bacc as bacc
    import concourse.bass as bass
    import concourse.tile as tile
    import concourse.mybir as mybir

    f32 = mybir.dt.float32
    bf16 = mybir.dt.float16
    i16 = mybir.dt.int16
    AF = mybir.ActivationFunctionType
    OP = mybir.AluOpType
    core_ids = list(range(NC))
    NTBMAX = int(max(n_tiles))
    # L1 blocks whose one-hot tiles stream from DRAM (rest: DVE is_equal)
    soff = {}
    ts = 0
    for b in range(NB):
        if b in STREAM1:
            soff[b] = ts
            ts += int(n_tiles[b])
    TS = max(ts, 1)

    nc = bacc.Bacc(None, target_bir_lowering=False, debug=False,
                   dynamic_dma_scratch_size=SCRATCH, num_swdge_queues=4)

    # ---- I/O ----
    fexp_d = nc.dram_tensor("fexp", [P, T * DIN], bf16, kind="ExternalInput")
    gidx_d = nc.dram_tensor("gidx", [P, T * 8], i16, kind="ExternalInput")
    dloc_d = nc.dram_tensor("dloc", [P, T], bf16, kind="ExternalInput")
    inv2_d = nc.dram_tensor("inv2", [P, NB], f32, kind="ExternalInput")
    fown_d = nc.dram_tensor("fown", [P, NB * DIN], bf16, kind="ExternalInput")
    w1_d = nc.dram_tensor("w1", [DIN, DH], bf16, kind="ExternalInput")
    w2_d = nc.dram_tensor("w2", [DH, DH], bf16, kind="ExternalInput")
    wm1_d = nc.dram_tensor("wm1", [DH, MH], bf16, kind="ExternalInput")
    b1_d = nc.dram_tensor("b1c", [DH, 1], f32, kind="ExternalInput")
    b2_d = nc.dram_tensor("b2c", [DH, 1], f32, kind="ExternalInput")
    # packed per-partition column params:
    # 0=bm1[:128] 1=bm1[128:] 2=gamma[:128] 3=gamma[128:]
    # 4=beta[:128] 5=beta[128:] 6=wm2[:128] 7=wm2[128:]
    bnpk_d = nc.dram_tensor("bnpk", [P, 8], f32, kind="ExternalInput")
    bm2_d = nc.dram_tensor("bm2s", [1, 1], f32, kind="ExternalInput")
    iota_d = nc.dram_tensor("iota", [P, P], bf16, kind="ExternalInput")
    identb_d = nc.dram_tensor("identb", [P, P], bf16, kind="ExternalInput")
    ident2_d = nc.dram_tensor("ident2", [P, P], bf16, kind="ExternalInput")
    ohs_d = nc.dram_tensor("ohs", [P, TS * P], bf16, kind="ExternalInput")
    identf_d = nc.dram_tensor("identf", [P, P], f32, kind="ExternalInput")
    out_d = nc.dram_tensor("out", [1, NPAD], f32, kind="ExternalOutput")

    # internal DRAM
    sliceA = nc.dram_tensor("sliceA", [RSPLIT, DH], bf16)
    sliceB = nc.dram_tensor("sliceB", [BSZ, DH], bf16)
    h1A = nc.dram_tensor("h1A", [NHALF_A, DH], bf16, addr_space="Shared")
    h1B = nc.dram_tensor("h1B", [NHALF_B, DH], bf16, addr_space="Shared")
    stats_in = nc.dram_tensor("stats_in", [P, 4], f32)
    stats_out = nc.dram_tensor("stats_out", [P, 4], f32, addr_space="Shared")

    with tile.TileContext(nc) as tc:
        with tc.tile_pool(name="persist", bufs=1) as pp, \
             tc.tile_pool(name="fstream", bufs=FSPBUFS) as fsp, \
             tc.tile_pool(name="ohpool", bufs=OHPBUFS) as ohp, \
             tc.tile_pool(name="gpool", bufs=GSPBUFS) as gsp, \
             tc.tile_pool(name="tmp", bufs=3) as tp, \
             tc.tile_pool(name="pagg", bufs=2, space="PSUM") as pagg, \
             tc.tile_pool(name="ptrp", bufs=2, space="PSUM") as ptrp, \
             tc.tile_pool(name="pwz", bufs=2, space="PSUM") as pwz, \
             tc.tile_pool(name="pfin", bufs=2, space="PSUM") as pfin:

            # ---- persistent tiles ----
            gidx_t = pp.tile([P, T * 8], i16)
            dloc_t = pp.tile([P, T], bf16)
            inv2_t = pp.tile([P, NB], f32)
            fown_t = pp.tile([P, NB, DIN], bf16)
            w1_t = pp.tile([DIN, DH], bf16)
            w2_t = pp.tile([DH, DH], bf16)
            wm1_t = pp.tile([DH, MH], bf16)
            b1_t = pp.tile([DH, 1], f32)
            b2_t = pp.tile([DH, 1], f32)
            bnpk_t = pp.tile([P, 8], f32)
            iota_t = pp.tile([P, P], bf16)
            identb_t = pp.tile([P, P], bf16)
            ident2_t = pp.tile([P, P], bf16)
            identf_t = pp.tile([P, P], f32)
            h1own_t = pp.tile([P, NB, DH], bf16)
            zT1_t = pp.tile([P, NB, P], bf16)
            zT2_t = pp.tile([MH2, NB, P], bf16)
            stacc_t = pp.tile([P, 4], f32)
            eps_t = pp.tile([P, 1], f32)
            invN_t = pp.tile([P, 1], f32)
            nc.vector.memset(eps_t[:], EPS)
            nc.vector.memset(invN_t[:], 1.0 / N)
            nc.vector.memset(stacc_t[:], 0.0)

            nc.sync.dma_start(gidx_t[:], gidx_d[:])
            nc.sync.dma_start(dloc_t[:], dloc_d[:])
            nc.sync.dma_start(inv2_t[:], inv2_d[:])
            nc.scalar.dma_start(fown_t[:], fown_d.rearrange(
                "p (b d) -> p b d", d=DIN))
            nc.sync.dma_start(w1_t[:], w1_d[:])
            nc.sync.dma_start(w2_t[:], w2_d[:])
            nc.sync.dma_start(wm1_t[:], wm1_d[:])
            nc.sync.dma_start(b1_t[:], b1_d[:])
            nc.sync.dma_start(b2_t[:], b2_d[:])
            nc.sync.dma_start(bnpk_t[:], bnpk_d[:])
            nc.sync.dma_start(iota_t[:], iota_d[:])
            nc.sync.dma_start(identb_t[:], identb_d[:])
            nc.sync.dma_start(ident2_t[:], ident2_d[:])
            nc.sync.dma_start(identf_t[:], identf_d[:])

            fexp_r = fexp_d.rearrange("p (t d) -> p t d", d=DIN)
            ohs_r = ohs_d.rearrange("p (t j) -> p t j", j=P)

            def onehot(b, eng):
                t0, t1 = int(tob[b]), int(tob[b + 1])
                ntb = t1 - t0
                oh = ohp.tile([P, NTBMAX, P], bf16, tag="oh")
                eng.tensor_tensor(
                    out=oh[:, :ntb, :],
                    in0=dloc_t[:, t0:t1].unsqueeze(2).to_broadcast(
                        [P, ntb, P]),
                    in1=iota_t[:].unsqueeze(1).to_broadcast([P, ntb, P]),
                    op=OP.is_equal)
                return oh

            # ---- layer 1: stream pre-gathered features ----
            for b in range(NB):
                rows_b = P if b < NB - 1 else LAST_ROWS
                t0, t1 = int(tob[b]), int(tob[b + 1])
                ntb = t1 - t0
                rt = fsp.tile([P, NTBMAX, DIN], bf16, tag="ft")
                deng = nc.sync if b % 2 == 0 else nc.scalar
                deng.dma_start(rt[:, :ntb, :], fexp_r[:, t0:t1, :])
                if b in STREAM1:
                    oh = ohp.tile([P, NTBMAX, P], bf16, tag="oh")
                    oeng = nc.scalar if b % 2 == 0 else nc.sync
                    oeng.dma_start(oh[:, :ntb, :],
                                   ohs_r[:, soff[b]:soff[b] + ntb, :])
                else:
                    oh = onehot(b, nc.vector)
                pm = pagg.tile([P, DH], f32, tag="pm")
                for ti in range(ntb):
                    nc.tensor.matmul(out=pm[:, :DIN], lhsT=oh[:, ti, :],
                                     rhs=rt[:, ti, :],
                                     start=(ti == 0), stop=False)
                nc.tensor.matmul(out=pm[:, :DIN], lhsT=ident2_t[:],
                                 rhs=fown_t[:, b, :], start=False, stop=True)
                hn = tp.tile([P, DIN], bf16, tag="hn")
                nc.scalar.activation(hn[:], pm[:, :DIN], AF.Copy,
                                     scale=inv2_t[:, b:b + 1])
                ptt = ptrp.tile([P, P], bf16, tag="ptt")
                nc.tensor.transpose(out=ptt[:DIN, :], in_=hn[:],
                                    identity=identb_t[:])
                hnT = tp.tile([DIN, P], bf16, tag="hnT")
                nc.scalar.activation(hnT[:], ptt[:DIN, :], AF.Copy)
                pww = pwz.tile([P, MH + P], f32, tag="pwz")
                nc.tensor.matmul(out=pww[:, MH:], lhsT=w1_t[:], rhs=hnT[:],
                                 start=True, stop=True)
                hT = tp.tile([DH, P], bf16, tag="hT")
                nc.scalar.activation(hT[:], pww[:, MH:], AF.Relu,
                                     bias=b1_t[:])
                pt2 = ptrp.tile([P, P], bf16, tag="ptt")
                nc.tensor.transpose(out=pt2[:], in_=hT[:],
                                    identity=identb_t[:])
                nc.scalar.activation(h1own_t[:, b, :], pt2[:], AF.Copy)
                # rows [b*128, b*128+rows_b) of the slice, split at RSPLIT
                lo = b * P
                hi = lo + rows_b
                if hi <= RSPLIT:
                    nc.sync.dma_start(sliceA[lo:hi, :],
                                      h1own_t[:rows_b, b, :])
                elif lo >= RSPLIT:
                    nc.sync.dma_start(sliceB[lo - RSPLIT:hi - RSPLIT, :],
                                      h1own_t[:rows_b, b, :])
                else:
                    cut = RSPLIT - lo
                    nc.sync.dma_start(sliceA[lo:RSPLIT, :],
                                      h1own_t[:cut, b, :])
                    nc.sync.dma_start(sliceB[0:hi - RSPLIT, :],
                                      h1own_t[cut:rows_b, b, :])
                if hi >= RSPLIT and lo < RSPLIT:
                    # first half of every core's slice is complete
                    nc.gpsimd.collective_compute(
                        "AllGather", mybir.AluOpType.bypass,
                        replica_groups=[core_ids],
                        ins=[sliceA[:]], outs=[h1A[:]],
                    )

            # ---- layer 2: gather h1 rows, aggregate, fused MLP hidden ----
            rts = {}
            gq = [0]

            def gather_seg(b, s0, s1, tab):
                rt = rts[b]
                t0 = int(tob[b])
                nt = s1 - s0
                k = (nt + GCHUNK - 1) // GCHUNK
                base, rem = divmod(nt, k)
                c0 = s0
                for j in range(k):  # balanced chunk sizes
                    c1 = c0 + base + (1 if j < rem else 0)
                    ni = (c1 - c0) * P
                    nc.gpsimd.dma_gather(
                        out_ap=rt[:, c0:c1, :], in_ap=tab,
                        idxs_ap=gidx_t[:, 8 * (t0 + c0):8 * (t0 + c1)],
                        num_idxs=ni, num_idxs_reg=ni,
                        elem_size=DH,
                        queue_num=gq[0] % 4,
                    )
                    gq[0] += 1
                    c0 = c1

            def gather_a(b):
                rts[b] = gsp.tile([P, NTBMAX, DH], bf16, tag="gt",
                                  name=f"rt{b}")
                gather_seg(b, 0, int(ntA[b]), h1A[:])

            def gather_b(b):
                t0, t1 = int(tob[b]), int(tob[b + 1])
                gather_seg(b, int(ntA[b]), t1 - t0, h1B[:])

            # A-prefetch issues as soon as AG1 lands (gpsimd idle in L1);
            # the AG2 trigger goes after a few A-gathers: late enough not
            # to stall them, early enough to fire soon after sliceB lands
            for b in range(min(PF_A, NB)):
                if b == 4:
                    nc.gpsimd.collective_compute(
                        "AllGather", mybir.AluOpType.bypass,
                        replica_groups=[core_ids],
                        ins=[sliceB[:]], outs=[h1B[:]],
                    )
                gather_a(b)
            for b in range(min(PF_B, NB)):
                gather_b(b)

            for b in range(NB):
                if b + PF_B < NB:
                    gather_b(b + PF_B)
                if b + PF_A < NB:
                    gather_a(b + PF_A)
                t0, t1 = int(tob[b]), int(tob[b + 1])
                ntb = t1 - t0
                rt = rts.pop(b)
                oh = onehot(b, nc.vector)
                pm = pagg.tile([P, DH], f32, tag="pm")
                for ti in range(ntb):
                    nc.tensor.matmul(out=pm[:], lhsT=oh[:, ti, :],
                                     rhs=rt[:, ti, :],
                                     start=(ti == 0), stop=False)
                nc.tensor.matmul(out=pm[:], lhsT=ident2_t[:],
                                 rhs=h1own_t[:, b, :], start=False, stop=True)
                hn = tp.tile([P, DH], bf16, tag="hn2")
                nc.scalar.activation(hn[:], pm[:], AF.Copy,
                                     scale=inv2_t[:, b:b + 1])
                ptt = ptrp.tile([P, P], bf16, tag="ptt")
                nc.tensor.transpose(out=ptt[:], in_=hn[:],
                                    identity=identb_t[:])
                hnT = tp.tile([DH, P], bf16, tag="hnT2")
                nc.scalar.activation(hnT[:], ptt[:], AF.Copy)
                pww = pwz.tile([P, MH + P], f32, tag="pwz")
                nc.tensor.matmul(out=pww[:, MH:], lhsT=w2_t[:], rhs=hnT[:],
                                 start=True, stop=True)
                h2T = tp.tile([DH, P], bf16, tag="h2T")
                nc.scalar.activation(h2T[:], pww[:, MH:], AF.Relu,
                                     bias=b2_t[:])
                # fused MLP hidden in transposed layout:
                # zT[m, p] = relu(sum_d wm1[d, m] h2T[d, p] + bm1[m])
                pz = pwz.tile([P, MH + P], f32, tag="pwz")
                nc.tensor.matmul(out=pz[:, :P], lhsT=wm1_t[:, :P],
                                 rhs=h2T[:], start=True, stop=True)
                nc.tensor.matmul(out=pz[:MH2, P:2 * P], lhsT=wm1_t[:, P:],
                                 rhs=h2T[:], start=True, stop=True)
                nc.scalar.activation(zT1_t[:, b, :], pz[:, :P], AF.Relu,
                                     bias=bnpk_t[:, 0:1])
                nc.scalar.activation(zT2_t[:, b, :], pz[:MH2, P:2 * P],
                                     AF.Relu, bias=bnpk_t[:MH2, 1:2])
                if b == NB - 1:
                    # zero pad columns so batch stats stay clean
                    nc.vector.memset(zT1_t[:, b, LAST_ROWS:], 0.0)
                    nc.vector.memset(zT2_t[:, b, LAST_ROWS:], 0.0)
                # incremental batch stats: sum(z), sum(z^2) over this block
                sqb = tp.tile([P, 2 * P], bf16, tag="sqb")
                nc.scalar.activation(sqb[:, :P], zT1_t[:, b, :], AF.Square)
                nc.scalar.activation(sqb[:MH2, P:], zT2_t[:, b, :],
                                     AF.Square)
                rb = tp.tile([P, 4], f32, tag="rb")
                nc.vector.tensor_reduce(out=rb[:, 0:1], in_=zT1_t[:, b, :],
                                        axis=mybir.AxisListType.X, op=OP.add)
                nc.vector.tensor_reduce(out=rb[:, 1:2], in_=sqb[:, :P],
                                        axis=mybir.AxisListType.X, op=OP.add)
                nc.vector.tensor_reduce(out=rb[:MH2, 2:3],
                                        in_=zT2_t[:, b, :],
                                        axis=mybir.AxisListType.X, op=OP.add)
                nc.vector.tensor_reduce(out=rb[:MH2, 3:4],
                                        in_=sqb[:MH2, P:],
                                        axis=mybir.AxisListType.X, op=OP.add)
                nc.vector.tensor_tensor(out=stacc_t[:, 0:2],
                                        in0=stacc_t[:, 0:2],
                                        in1=rb[:, 0:2], op=OP.add)
                nc.vector.tensor_tensor(out=stacc_t[:MH2, 2:4],
                                        in0=stacc_t[:MH2, 2:4],
                                        in1=rb[:MH2, 2:4], op=OP.add)

            nc.sync.dma_start(stats_in[:], stacc_t[:])
            nc.gpsimd.collective_compute(
                "AllReduce", mybir.AluOpType.add,
                replica_groups=[core_ids],
                ins=[stats_in[:]], outs=[stats_out[:]],
            )
            gst = tp.tile([P, 4], f32, tag="gst")
            nc.sync.dma_start(gst[:], stats_out[:])

            # ---- fold BN into the final matvec (per-partition columns) ----
            mu = tp.tile([P, 2], f32, tag="mu")
            var = tp.tile([P, 2], f32, tag="var")
            scl = tp.tile([P, 2], f32, tag="scl")
            wp = tp.tile([P, 2], bf16, tag="wp")
            ws = tp.tile([P, 2], f32, tag="ws")
            nc.vector.tensor_tensor(out=mu[:, 0:1], in0=gst[:, 0:1],
                                    in1=invN_t[:], op=OP.mult)
            nc.vector.tensor_tensor(out=mu[:, 1:2], in0=gst[:, 2:3],
                                    in1=invN_t[:], op=OP.mult)
            nc.vector.tensor_tensor(out=var[:, 0:1], in0=gst[:, 1:2],
                                    in1=invN_t[:], op=OP.mult)
            nc.vector.tensor_tensor(out=var[:, 1:2], in0=gst[:, 3:4],
                                    in1=invN_t[:], op=OP.mult)
            mu2 = tp.tile([P, 2], f32, tag="mu2")
            nc.vector.tensor_tensor(out=mu2[:], in0=mu[:], in1=mu[:],
                                    op=OP.mult)
            nc.vector.tensor_tensor(out=var[:], in0=var[:], in1=mu2[:],
                                    op=OP.subtract)
            nc.scalar.activation(var[:], var[:], AF.Sqrt, bias=eps_t[:])
            rstd = tp.tile([P, 2], f32, tag="rstd")
            nc.vector.reciprocal(rstd[:], var[:])
            # scale = gamma * rstd ; shift = beta - mu * scale
            nc.vector.tensor_tensor(out=scl[:, 0:1], in0=bnpk_t[:, 2:3],
                                    in1=rstd[:, 0:1], op=OP.mult)
            nc.vector.tensor_tensor(out=scl[:, 1:2], in0=bnpk_t[:, 3:4],
                                    in1=rstd[:, 1:2], op=OP.mult)
            msc = tp.tile([P, 2], f32, tag="msc")
            nc.vector.tensor_tensor(out=msc[:], in0=mu[:], in1=scl[:],
                                    op=OP.mult)
            shf = tp.tile([P, 2], f32, tag="shf")
            nc.vector.tensor_tensor(out=shf[:, 0:1], in0=bnpk_t[:, 4:5],
                                    in1=msc[:, 0:1], op=OP.subtract)
            nc.vector.tensor_tensor(out=shf[:, 1:2], in0=bnpk_t[:, 5:6],
                                    in1=msc[:, 1:2], op=OP.subtract)
            # w' = wm2 * scale (bf16 for matmul); ws = wm2 * shift (f32)
            wpf = tp.tile([P, 2], f32, tag="wpf")
            nc.vector.tensor_tensor(out=wpf[:, 0:1], in0=bnpk_t[:, 6:7],
                                    in1=scl[:, 0:1], op=OP.mult)
            nc.vector.tensor_tensor(out=wpf[:, 1:2], in0=bnpk_t[:, 7:8],
                                    in1=scl[:, 1:2], op=OP.mult)
            nc.scalar.activation(wp[:], wpf[:], AF.Copy)
            nc.vector.tensor_tensor(out=ws[:, 0:1], in0=bnpk_t[:, 6:7],
                                    in1=shf[:, 0:1], op=OP.mult)
            nc.vector.tensor_tensor(out=ws[:, 1:2], in0=bnpk_t[:, 7:8],
                                    in1=shf[:, 1:2], op=OP.mult)
            # (rows MH2: of ws lane 2 are zero by construction: bnpk pads)
            # b' = sum_m ws + bm2 : transpose columns to rows, reduce twice
            pts = pfin.tile([P, 4 * P], f32, tag="pfin")
            nc.tensor.transpose(out=pts[:2, :P], in_=ws[:],
                                identity=identf_t[:])
            wsrow = tp.tile([2, P], f32, tag="wsrow")
            nc.scalar.activation(wsrow[:], pts[:2, :P], AF.Copy)
            ssum = tp.tile([2, 1], f32, tag="ssum")
            nc.vector.tensor_reduce(out=ssum[:], in_=wsrow[:],
                                    axis=mybir.AxisListType.X, op=OP.add)
            pts2 = pfin.tile([P, 4 * P], f32, tag="pfin")
            nc.tensor.transpose(out=pts2[:1, :2], in_=ssum[:],
                                identity=identf_t[:2, :2])
            ssrow = tp.tile([1, 2], f32, tag="ssrow")
            nc.scalar.activation(ssrow[:], pts2[:1, :2], AF.Copy)
            tot = tp.tile([1, 1], f32, tag="tot")
            nc.vector.tensor_reduce(out=tot[:], in_=ssrow[:],
                                    axis=mybir.AxisListType.X, op=OP.add)
            bm2_t = tp.tile([1, 1], f32, tag="bm2t")
            nc.sync.dma_start(bm2_t[:], bm2_d[:])
            bpr = tp.tile([1, 1], f32, tag="bpr")
            nc.vector.tensor_tensor(out=bpr[:], in0=tot[:], in1=bm2_t[:],
                                    op=OP.add)

            # ---- final: sigmoid(w1'.zT1 + w2'.zT2 + b') on tensor eng ----
            zT1f = zT1_t[:].rearrange("m b p -> m (b p)")
            zT2f = zT2_t[:].rearrange("m b p -> m (b p)")
            CG = 4 * P  # 512 output columns per group
            ngrp = (NPAD + CG - 1) // CG
            for g in range(ngrp):
                c0 = g * CG
                c1 = min(c0 + CG, NPAD)
                cw = c1 - c0
                po = pfin.tile([P, 4 * P], f32, tag="pfin")
                nc.tensor.matmul(out=po[0:1, :cw], lhsT=wp[:, 0:1],
                                 rhs=zT1f[:, c0:c1], start=True, stop=False)
                nc.tensor.matmul(out=po[0:1, :cw], lhsT=wp[:MH2, 1:2],
                                 rhs=zT2f[:, c0:c1], start=False, stop=True)
                orow = tp.tile([1, CG], f32, tag="orow")
                nc.scalar.activation(orow[0:1, :cw], po[0:1, :cw],
                                     AF.Sigmoid, bias=bpr[:])
                nc.sync.dma_start(out_d[0:1, c0:c1], orow[0:1, :cw])

    nc.compile()
    return nc


# module-level cache of (program, layout) keyed by edge-structure hash
_CACHE = {}


def kernel(features, W1, b1, W2, b2, Wm1, bm1, gamma, beta, Wm2, bm2, src, dst):
    from concourse.bass_utils import run_bass_kernel_spmd

    features = np.asarray(features, np.float32)
    src = np.asarray(src, np.int64)
    dst = np.asarray(dst, np.int64)

    key = (int(src[:1000].sum()), int(dst[:1000].sum()), E)
    if key not in _CACHE:
        n_tiles, ntA, tob, T, gidx, gnode, dlocT = _build_edge_layout(src, dst)
        nc = build_program(n_tiles, ntA, tob, T)
        _CACHE[key] = (nc, n_tiles, ntA, tob, T, gidx, gnode, dlocT)
    nc, n_tiles, ntA, tob, T, gidx, gnode, dlocT = _CACHE[key]

    deg = np.bincount(dst, minlength=N).astype(np.float32)
    inv2 = (1.0 / (deg + 2.0)).astype(np.float32)
    features_bf = features.astype(BF16)

    iota = np.tile(np.arange(P, dtype=np.float32), (P, 1)).astype(BF16)
    identb = np.eye(P, dtype=np.float32).astype(BF16)
    ident2 = (2.0 * np.eye(P, dtype=np.float32)).astype(BF16)
    identf = np.eye(P, dtype=np.float32)
    jcols = np.arange(P, dtype=np.float32)

    W1b = np.asarray(W1, np.float32).astype(BF16)
    W2b = np.asarray(W2, np.float32).astype(BF16)
    Wm1b = np.asarray(Wm1, np.float32).astype(BF16)
    bm1f = np.asarray(bm1, np.float32).reshape(MH)
    gamf = np.asarray(gamma, np.float32).reshape(MH)
    betf = np.asarray(beta, np.float32).reshape(MH)
    wm2f = np.asarray(Wm2, np.float32).reshape(MH)
    bnpk = np.zeros((P, 8), np.float32)
    for i, v in enumerate((bm1f, gamf, betf, wm2f)):
        bnpk[:, 2 * i] = v[:P]
        bnpk[:MH2, 2 * i + 1] = v[P:]

    in_maps = []
    for c in range(NC):
        lo = c * NLOC
        fexp = features_bf[gnode[c]].reshape(P, T * DIN)
        # dma_gather int16 indices: position i=t*128+p at [16-wrap], x8 replicas
        flat = gidx[c].astype(np.int64).T.reshape(-1)  # i = t*128+p
        wrapped = flat.reshape(T * 8, 16).T.astype(np.int16)  # [16, T*8]
        gidx16 = np.ascontiguousarray(np.tile(wrapped, (8, 1)))  # [128, T*8]
        inv2p = np.zeros(NPAD, np.float32)
        inv2p[:NLOC] = inv2[lo:lo + NLOC]
        inv2T = np.ascontiguousarray(inv2p.reshape(NB, P).T)
        fownp = np.zeros((NPAD, DIN), BF16)
        fownp[:NLOC] = features_bf[lo:lo + NLOC]
        fown = np.ascontiguousarray(
            fownp.reshape(NB, P, DIN).transpose(1, 0, 2).reshape(P, NB * DIN))
        oh_parts = []
        for b in range(NB):
            if b in STREAM1:
                t0, t1 = int(tob[b]), int(tob[b + 1])
                ohb = (dlocT[c][:, t0:t1, None] == jcols[None, None, :])
                oh_parts.append(ohb.astype(BF16).reshape(P, -1))
        if oh_parts:
            ohs = np.ascontiguousarray(np.concatenate(oh_parts, axis=1))
        else:
            ohs = np.zeros((P, P), BF16)

        in_maps.append({
            "fexp": np.ascontiguousarray(fexp),
            "gidx": gidx16,
            "dloc": np.ascontiguousarray(dlocT[c].astype(BF16)),
            "inv2": inv2T,
            "fown": fown,
            "ohs": ohs,
            "w1": W1b,
            "w2": W2b,
            "wm1": Wm1b,
            "b1c": np.asarray(b1, np.float32).reshape(DH, 1),
            "b2c": np.asarray(b2, np.float32).reshape(DH, 1),
            "bnpk": bnpk,
            "bm2s": np.asarray(bm2, np.float32).reshape(1, 1),
            "iota": iota,
            "identb": identb,
            "ident2": ident2,
            "identf": identf,
        })

    res = run_bass_kernel_spmd(nc, in_maps, list(range(NC)))
    global _LAST
    _LAST = res
    out = np.concatenate(
        [res.results[c]["out"].reshape(-1)[:NLOC] for c in range(NC)], axis=0)
    return out.reshape(N, 1).astype(np.float32)


_LAST = None
